# revision 6
# baseline (speedup 1.0000x reference)
"""Trainium2 Bass kernel for nn_AdaptiveAdjacencyMatrix.

Reference math:
    s[b, i]        = sum_d h[b, i, d] * w[d]
    scores[b,i,j]  = s[b,i] + s[b,j] + bias
    A              = softmax(scores, axis=1)   # over i

Because the softmax is over axis=1 (i), the `s[b,j] + bias` term is constant
along the reduced axis and cancels exactly:
    A[b, i, j] = exp(s[b,i]) / sum_i' exp(s[b,i'])   (independent of j and bias)

So the output is a column-broadcast of softmax(s[b]) — the kernel is purely
memory-bound. The output is written in bf16 (upcast to f32 on the host),
which halves HBM write traffic: 2048*4096*2 = 16.8 MB per core, on top of
the 2 MB bf16 h read. The write stream sustains ~426 GB/s (8 KB
per-partition contiguous descriptors), so the stream floor is ~39 us; the
rest is head (preamble + h load + dot products + softmax tail).

Sharding: 8 cores = (batch b, row-half rh). Each core receives the full
h[b] (rows reordered so its own 2048 rows come first), computes softmax(s)
locally (the softmax sum needs all 4096 rows anyway; row order is
irrelevant to the sum), and writes a [2048, 4096] output shard. No
collectives needed.

Head structure:
  - w rides the scalar (ACT) ring so the sync ring's FIFO starts with h;
    the ACT table set containing Exp is pre-warmed under the DMA shadow.
  - h arrives as two 1 MB DMAs (other half first, since its exp can
    overlap the own-half dot products) with 8 KB per-partition contiguous
    descriptors.
  - Dot products: DVE multiplies 4-row slices (bf16 2x mode) and reduces
    three slices per half; ACT accum-reduces the first slice of each half
    (its per-row cost is ~3.9x DVE's, so it only gets one). The DVE
    instruction order interleaves mine-mults ahead of oth-reduces so ACT's
    mine-slice starts early. NOTE: tensor_tensor_reduce is NOT used — it
    crashes the exec unit on this HW (NRT_EXEC_UNIT_UNRECOVERABLE)
    despite passing CoreSim.
  - The first output group is filled/DMA'd in two half-column pieces so
    the write stream starts as early as possible; the last group is also
    split so the final DMA's completion receipt covers only 0.5 MB.

Layouts: h is DMA'd with fully-contiguous per-partition descriptors
(partition p holds rows 16p..16p+15 of its half), which makes the on-chip
softmax come out in a (q, r) layout where device output row 128r + q holds
the value for local input row 16q + r. The host unshard undoes that with a
cheap reshape/transpose (and upcasts bf16 -> f32).
"""

import ml_dtypes
import numpy as np

B, N, D = 4, 4096, 256
NCORES = 8
HALF = N // 2          # 2048 rows written per core
P = 128                # SBUF partitions
RPP = HALF // P        # 16 rows per partition (per half)
SL = 4                 # rows per dot-product slice
SUP = 2                # groups per output supertile / DMA (2 MB in bf16)

_CACHE = {}


def _build():
    import concourse.mybir as mybir
    import concourse.tile as tile
    from concourse import bacc

    f32 = mybir.dt.float32
    Copy = mybir.ActivationFunctionType.Copy
    Exp = mybir.ActivationFunctionType.Exp
    nc = bacc.Bacc("TRN2", target_bir_lowering=False, debug=False)

    bf16 = mybir.dt.bfloat16
    h_ext = nc.declare_dram_parameter("h", [N, D], bf16, isOutput=False)
    # w arrives pre-broadcast to 128 partitions (host-side tile of the 1 KB
    # vector); it rides the scalar ring so h owns the sync ring FIFO.
    w_ext = nc.declare_dram_parameter("wb", [P, D], bf16, isOutput=False)
    out_ext = nc.declare_dram_parameter("out", [HALF, N], bf16, isOutput=True)

    # contiguous flat views: partition p holds rows 16p..16p+15 of each half
    h_mine = h_ext[0:HALF, :].rearrange("(p r) d -> p r d", p=P)
    h_oth = h_ext[HALF:N, :].rearrange("(p r) d -> p r d", p=P)
    # [128, r, j] view of out: device out row = 128r + q (host un-permutes)
    out_r = out_ext[:, :].rearrange("(r q) j -> q r j", q=P)

    n_sl = RPP // SL  # 4 slices per half

    with tile.TileContext(nc) as tc:
        with (
            tc.tile_pool(name="const", bufs=1) as cpool,
            tc.tile_pool(name="hload", bufs=2) as hpool,
            tc.tile_pool(name="prod", bufs=2 * n_sl) as ppool,
            tc.tile_pool(name="small", bufs=1) as spool,
            tc.tile_pool(name="obuf", bufs=4) as opool,
            tc.tile_pool(name="psum", bufs=1, space="PSUM") as psum_pool,
        ):
            # all-ones [128,128] for the PE cross-partition-sum trick
            ones_k = cpool.tile([P, P], f32)
            nc.vector.memset(ones_k[:, :], 1.0)
            # all-ones [128, 4096] bf16: DVE output fills become
            # tensor_scalar multiplies with real strides
            ones_4k = cpool.tile([P, N], bf16)
            nc.vector.memset(ones_4k[:, :], 1.0)

            # Warm the ACT table set that contains Exp before any data
            # arrives: the ~2.7us ACT_TABLE_LOAD overlaps the DMAs instead
            # of landing in the critical softmax tail.
            warm = spool.tile([P, 1], f32)
            nc.vector.memset(warm[:, 0:1], 0.0)
            warm2 = spool.tile([P, 1], f32)

            # --- DMAs: w on the scalar ring; h (other half first, then
            # mine) as two 1 MB transfers on the sync ring ---
            w_bc = cpool.tile([P, D], bf16)
            nc.scalar.dma_start(out=w_bc[:, :], in_=w_ext[:, :])
            nc.scalar.activation(out=warm2[:, 0:1], in_=warm[:, 0:1], func=Exp)
            # materialized repeat (real strides for the DVE 2x bf16 mode)
            w_rep = cpool.tile([P, SL, D], bf16)
            nc.vector.tensor_copy(
                w_rep[:, :, :],
                w_bc[:, :].unsqueeze(1).broadcast_to([P, SL, D]),
            )

            h_oth_t = hpool.tile([P, RPP, D], bf16)
            nc.sync.dma_start(out=h_oth_t[:, :, :], in_=h_oth[:, :, :])
            h_mine_t = hpool.tile([P, RPP, D], bf16)
            nc.sync.dma_start(out=h_mine_t[:, :, :], in_=h_mine[:, :, :])

            s_mine = spool.tile([P, RPP], f32)
            s_oth = spool.tile([P, RPP], f32)
            e_mine = spool.tile([P, RPP], f32)
            jnk_e = spool.tile([P, RPP], f32)
            jnk = spool.tile([P, D], f32)
            rs_m = spool.tile([P, 1], f32)
            rs_o = spool.tile([P, 1], f32)
            tot_psum = psum_pool.tile([P, 1], f32)

            # dot products: DVE multiplies every slice; slice 0 of each half
            # reduces on ACT (copy+accum per row), slices 1..3 on DVE.
            # DVE program order interleaves mine's first mult right after
            # oth's mults so ACT's mine-slice isn't starved.
            prods = {}

            def mult(half_t, sl):
                pr = ppool.tile([P, SL, D], bf16)
                nc.vector.tensor_tensor(
                    out=pr[:, :, :],
                    in0=half_t[:, sl * SL : (sl + 1) * SL, :],
                    in1=w_rep[:, :, :],
                    op=mybir.AluOpType.mult,
                )
                prods[(id(half_t), sl)] = pr
                return pr

            def red_dve(half_t, s_dst, sl):
                pr = prods[(id(half_t), sl)]
                nc.vector.tensor_reduce(
                    out=s_dst[:, sl * SL : (sl + 1) * SL],
                    in_=pr[:, :, :],
                    axis=mybir.AxisListType.X,
                    op=mybir.AluOpType.add,
                )

            def red_act(half_t, s_dst, sl):
                pr = prods[(id(half_t), sl)]
                for g in range(SL):
                    gi = sl * SL + g
                    nc.scalar.activation(
                        out=jnk[:, :],
                        in_=pr[:, g, :],
                        func=Copy,
                        accum_out=s_dst[:, gi : gi + 1],
                    )

            for sl in range(n_sl):
                mult(h_oth_t, sl)
            red_act(h_oth_t, s_oth, 0)          # ACT: oth slice 0
            mult(h_mine_t, 0)                   # DVE: early mine mult
            red_act(h_mine_t, s_mine, 0)        # ACT: mine slice 0
            for sl in range(1, n_sl):
                red_dve(h_oth_t, s_oth, sl)     # DVE: oth slices 1..3
            # oth half's exp + row-sum + its PE pass overlap mine's dots
            nc.scalar.activation(
                out=jnk_e[:, :], in_=s_oth[:, :], func=Exp,
                accum_out=rs_o[:, 0:1],
            )
            rs_o2 = spool.tile([P, 1], f32)
            nc.vector.tensor_copy(rs_o2[:, 0:1], rs_o[:, 0:1])
            nc.tensor.matmul(
                tot_psum[:, 0:1], ones_k[:, 0:P], rs_o2[:, 0:1],
                start=True, stop=False,
            )
            for sl in range(1, n_sl):
                mult(h_mine_t, sl)
            for sl in range(1, n_sl):
                red_dve(h_mine_t, s_mine, sl)   # DVE: mine slices 1..3

            # --- tail: exp(mine), second PE pass, reciprocal, p ---
            nc.scalar.activation(
                out=e_mine[:, :], in_=s_mine[:, :], func=Exp,
                accum_out=rs_m[:, 0:1],
            )
            rs_m2 = spool.tile([P, 1], f32)
            nc.vector.tensor_copy(rs_m2[:, 0:1], rs_m[:, 0:1])
            nc.tensor.matmul(
                tot_psum[:, 0:1], ones_k[:, 0:P], rs_m2[:, 0:1],
                start=False, stop=True,
            )
            inv = spool.tile([P, 1], f32)
            nc.vector.reciprocal(inv[:, 0:1], tot_psum[:, 0:1])
            # p = e * (1/S): f32 for DVE fills' scalar operand, bf16 for
            # ACT fills' broadcast source
            p_f32 = spool.tile([P, RPP], f32)
            nc.vector.tensor_scalar_mul(p_f32[:, :], e_mine[:, :], inv[:, 0:1])
            p_bf = spool.tile([P, RPP], bf16)
            nc.vector.tensor_copy(p_bf[:, :], p_f32[:, :])

            # --- output: group g is p[:, g] broadcast along 4096 columns.
            # DVE fills: ones_4k * p (tensor_scalar, real strides);
            # ACT fills: stride-0 broadcast copy of p_bf. First and last
            # groups go out as two half-column pieces (earlier stream
            # start / smaller final completion receipt). All output DMAs
            # ride the sync ring. ---
            def fill(dst, g, eng):
                if eng == "v":
                    nc.vector.tensor_scalar_mul(
                        dst, ones_4k[:, 0 : dst.shape[-1]], p_f32[:, g : g + 1]
                    )
                else:
                    nc.scalar.copy(
                        out=dst,
                        in_=p_bf[:, g : g + 1].broadcast_to(
                            [P, dst.shape[-1]]
                        ),
                    )

            JH = N // 2

            def dma_half(ot, g, piece):
                j0 = piece * JH
                nc.sync.dma_start(
                    out=out_r[:, g : g + 1, j0 : j0 + JH],
                    in_=ot[:, j0 : j0 + JH].rearrange(
                        "q (r j) -> q r j", r=1
                    ),
                )

            # group 0 in halves (DVE fills)
            ot0 = opool.tile([P, SUP * N], bf16, tag="ot")
            fill(ot0[:, 0:JH], 0, "v")
            dma_half(ot0, 0, 0)
            fill(ot0[:, JH:N], 0, "v")
            dma_half(ot0, 0, 1)
            # groups 1..14 in supertiles of 2 (odd ACT, even DVE)
            gi = 1
            while gi + SUP <= RPP - 1:
                ot = opool.tile([P, SUP * N], bf16, tag="ot")
                for g in range(SUP):
                    fill(ot[:, g * N : (g + 1) * N], gi + g,
                         "a" if (gi + g) % 2 else "v")
                nc.sync.dma_start(
                    out=out_r[:, gi : gi + SUP, :],
                    in_=ot[:, 0 : SUP * N].rearrange(
                        "q (r j) -> q r j", r=SUP
                    ),
                )
                gi += SUP
            # last group (15) in halves (ACT fills)
            otl = opool.tile([P, SUP * N], bf16, tag="ot")
            fill(otl[:, 0:JH], RPP - 1, "a")
            dma_half(otl, RPP - 1, 0)
            fill(otl[:, JH:N], RPP - 1, "a")
            dma_half(otl, RPP - 1, 1)
    nc.compile()
    return nc


def _get_nc():
    if "nc" not in _CACHE:
        _CACHE["nc"] = _build()
    return _CACHE["nc"]


def _ensure_axon_hooks():
    """bass_utils' trace path imports antenv.axon_hooks, which some images
    lack; provide a stub and register the real NTFF hook (via the boot
    shim's ctypes path) so tracing works instead of degrading."""
    try:
        import antenv.axon_hooks  # noqa: F401
        return
    except ImportError:
        pass
    import sys
    import types

    try:
        import antenv
    except ImportError:
        antenv = types.ModuleType("antenv")
        sys.modules["antenv"] = antenv
    m = types.ModuleType("antenv.axon_hooks")
    m._hook = None
    m.set_axon_ntff_profile_hook = lambda h: setattr(m, "_hook", h)
    m.get_axon_ntff_profile_hook = lambda: m._hook
    sys.modules["antenv.axon_hooks"] = m
    try:
        from trn_agent_boot.trn_boot import _ntff_profile_via_ctypes

        hk = _ntff_profile_via_ctypes("/opt/axon/libaxon_pjrt.so")
        if hk is not None:
            m._hook = hk
    except Exception:
        pass


def run_on_device(h, w, trace=False):
    """Run the SPMD kernel; returns the BassKernelResults."""
    from concourse.bass_utils import run_bass_kernel_spmd

    _ensure_axon_hooks()

    in_maps = []
    for c in range(NCORES):
        b_idx, rh = divmod(c, 2)
        hb = h[b_idx]
        if rh:
            hb = np.concatenate([hb[HALF:], hb[:HALF]], axis=0)
        in_maps.append(
            {
                "h": np.ascontiguousarray(hb.astype(ml_dtypes.bfloat16)),
                "wb": np.ascontiguousarray(
                    np.broadcast_to(w.astype(ml_dtypes.bfloat16), (P, D))
                ),
            }
        )
    res = run_bass_kernel_spmd(
        _get_nc(), in_maps, core_ids=list(range(NCORES)), trace=trace
    )
    return res


def kernel(h, w, b):
    h = np.asarray(h, dtype=np.float32)
    w = np.asarray(w, dtype=np.float32)
    res = run_on_device(h, w)
    A = np.empty((B, N, N), dtype=np.float32)
    for c in range(NCORES):
        b_idx, rh = divmod(c, 2)
        out_c = res.results[c]["out"]
        # device row 128r + q holds the value for local input row 16q + r:
        # undo with reshape/transpose (bf16 -> f32 upcast on assignment)
        unperm = (
            out_c.reshape(RPP, P, N).transpose(1, 0, 2).reshape(HALF, N)
        )
        A[b_idx, rh * HALF : (rh + 1) * HALF, :] = unperm
    return A


# revision 16
# speedup vs baseline: 1.0061x; 1.0061x over previous
"""Trainium2 Bass kernel for nn_AdaptiveAdjacencyMatrix.

Reference math:
    s[b, i]        = sum_d h[b, i, d] * w[d]
    scores[b,i,j]  = s[b,i] + s[b,j] + bias
    A              = softmax(scores, axis=1)   # over i

Because the softmax is over axis=1 (i), the `s[b,j] + bias` term is constant
along the reduced axis and cancels exactly:
    A[b, i, j] = exp(s[b,i]) / sum_i' exp(s[b,i'])   (independent of j and bias)

So the output is a column-broadcast of softmax(s[b]) — the kernel is purely
memory-bound. The output is written in bf16 (upcast to f32 on the host),
which halves HBM write traffic: 2048*4096*2 = 16.8 MB per core, on top of
the 2 MB bf16 h read. The write stream sustains ~426 GB/s (8 KB
per-partition contiguous descriptors), so the stream floor is ~39 us; the
rest is head (preamble + h load + dot products + softmax tail).

Sharding: 8 cores = (batch b, row-half rh). Each core receives the full
h[b] (rows reordered so its own 2048 rows come first), computes softmax(s)
locally (the softmax sum needs all 4096 rows anyway; row order is
irrelevant to the sum), and writes a [2048, 4096] output shard. No
collectives needed.

Head structure:
  - w rides the scalar (ACT) ring so the sync ring's FIFO starts with h;
    the ACT table set containing Exp is pre-warmed under the DMA shadow.
  - h arrives as two 1 MB DMAs (other half first, since its exp can
    overlap the own-half dot products) with 8 KB per-partition contiguous
    descriptors.
  - Dot products: DVE multiplies 4-row slices (bf16 2x mode) and reduces
    five of the eight slices; ACT accum-reduces three (oth slice 0, mine
    slices 0-1 — its per-row cost is ~2.6x DVE's). The DVE instruction
    order interleaves mine-mults ahead of oth-reduces so ACT's mine
    slices aren't starved. NOTE: tensor_tensor_reduce is NOT used — it
    crashes the exec unit on this HW (NRT_EXEC_UNIT_UNRECOVERABLE)
    despite passing CoreSim; GPSIMD tensor_reduce can't reduce the free
    axis.
  - The first output group is filled/DMA'd in two half-column pieces so
    the write stream starts as early as possible; the last group is also
    split so the final DMA's completion receipt covers only 0.5 MB.

Layouts: h is DMA'd with fully-contiguous per-partition descriptors
(partition p holds rows 16p..16p+15 of its half), which makes the on-chip
softmax come out in a (q, r) layout where device output row 128r + q holds
the value for local input row 16q + r. The host unshard undoes that with a
cheap reshape/transpose (and upcasts bf16 -> f32).
"""

import ml_dtypes
import numpy as np

B, N, D = 4, 4096, 256
NCORES = 8
HALF = N // 2          # 2048 rows written per core
P = 128                # SBUF partitions
RPP = HALF // P        # 16 rows per partition (per half)
SL = 4                 # rows per dot-product slice
SUP = 2                # groups per output supertile / DMA (2 MB in bf16)

_CACHE = {}


def _build():
    import concourse.mybir as mybir
    import concourse.tile as tile
    from concourse import bacc

    f32 = mybir.dt.float32
    Copy = mybir.ActivationFunctionType.Copy
    Exp = mybir.ActivationFunctionType.Exp
    nc = bacc.Bacc("TRN2", target_bir_lowering=False, debug=False)

    bf16 = mybir.dt.bfloat16
    h_ext = nc.declare_dram_parameter("h", [N, D], bf16, isOutput=False)
    # w arrives pre-broadcast/repeated to [128, SL, 256] on the host so the
    # DVE slice-multiplies read it with real strides and no on-chip repeat.
    w_ext = nc.declare_dram_parameter("wb", [P, SL, D], bf16, isOutput=False)
    out_ext = nc.declare_dram_parameter("out", [HALF, N], bf16, isOutput=True)

    # contiguous flat views: partition p holds rows 16p..16p+15 of each half
    h_mine = h_ext[0:HALF, :].rearrange("(p r) d -> p r d", p=P)
    h_oth = h_ext[HALF:N, :].rearrange("(p r) d -> p r d", p=P)
    # [128, r, j] view of out: device out row = 128r + q (host un-permutes)
    out_r = out_ext[:, :].rearrange("(r q) j -> q r j", q=P)

    n_sl = RPP // SL  # 4 slices per half

    with tile.TileContext(nc) as tc:
        with (
            tc.tile_pool(name="const", bufs=1) as cpool,
            tc.tile_pool(name="hload", bufs=2) as hpool,
            tc.tile_pool(name="prod", bufs=2 * n_sl) as ppool,
            tc.tile_pool(name="small", bufs=1) as spool,
            tc.tile_pool(name="obuf", bufs=4) as opool,
            tc.tile_pool(name="psum", bufs=1, space="PSUM") as psum_pool,
        ):
            # all-ones [128,128] for the PE cross-partition-sum trick
            ones_k = cpool.tile([P, P], f32)
            nc.vector.memset(ones_k[:, :], 1.0)
            # all-ones [128, 4096] bf16: DVE output fills become
            # tensor_scalar multiplies with real strides
            ones_4k = cpool.tile([P, N], bf16)
            nc.vector.memset(ones_4k[:, :], 1.0)

            # Warm the ACT table set that contains Exp before any data
            # arrives: the ~2.7us ACT_TABLE_LOAD overlaps the DMAs instead
            # of landing in the critical softmax tail.
            warm = spool.tile([P, 1], f32)
            nc.vector.memset(warm[:, 0:1], 0.0)
            warm2 = spool.tile([P, 1], f32)

            # --- DMAs, all on the sync ring: w (host-pretiled to
            # [P, SL, D] so no on-chip repeat is needed and its 2 KB
            # descriptors drain in ~0.8 us) first, then h (other half
            # first, then mine) as two 1 MB transfers. A scalar-ring w
            # was tried and starved behind the h DMAs' 8 KB packets
            # (round-robin at packet granularity) until ~12 us. ---
            nc.scalar.activation(out=warm2[:, 0:1], in_=warm[:, 0:1], func=Exp)
            w_rep = cpool.tile([P, SL, D], bf16)
            nc.sync.dma_start(out=w_rep[:, :, :], in_=w_ext[:, :, :])

            h_oth_t = hpool.tile([P, RPP, D], bf16)
            nc.sync.dma_start(out=h_oth_t[:, :, :], in_=h_oth[:, :, :])
            h_mine_t = hpool.tile([P, RPP, D], bf16)
            nc.sync.dma_start(out=h_mine_t[:, :, :], in_=h_mine[:, :, :])

            s_mine = spool.tile([P, RPP], f32)
            s_oth = spool.tile([P, RPP], f32)
            e_mine = spool.tile([P, RPP], f32)
            jnk_e = spool.tile([P, RPP], f32)
            jnk = spool.tile([P, D], f32)
            rs_m = spool.tile([P, 1], f32)
            rs_o = spool.tile([P, 1], f32)
            tot_psum = psum_pool.tile([P, 1], f32)

            # dot products: DVE multiplies every slice; reduces split
            # three ways per half — slice 0 on ACT (copy+accum per row),
            # slice 1 on GPSIMD, slices 2..3 on DVE. DVE program order
            # interleaves mine's early mults ahead of oth's reduces so
            # ACT's and GPSIMD's mine-slices aren't starved.
            prods = {}

            def mult(half_t, sl):
                pr = ppool.tile([P, SL, D], bf16)
                nc.vector.tensor_tensor(
                    out=pr[:, :, :],
                    in0=half_t[:, sl * SL : (sl + 1) * SL, :],
                    in1=w_rep[:, :, :],
                    op=mybir.AluOpType.mult,
                )
                prods[(id(half_t), sl)] = pr
                return pr

            def red_dve(half_t, s_dst, sl):
                pr = prods[(id(half_t), sl)]
                nc.vector.tensor_reduce(
                    out=s_dst[:, sl * SL : (sl + 1) * SL],
                    in_=pr[:, :, :],
                    axis=mybir.AxisListType.X,
                    op=mybir.AluOpType.add,
                )



            def red_act(half_t, s_dst, sl):
                pr = prods[(id(half_t), sl)]
                for g in range(SL):
                    gi = sl * SL + g
                    nc.scalar.activation(
                        out=jnk[:, :],
                        in_=pr[:, g, :],
                        func=Copy,
                        accum_out=s_dst[:, gi : gi + 1],
                    )

            for sl in range(n_sl):
                mult(h_oth_t, sl)               # DVE: oth mults
            red_act(h_oth_t, s_oth, 0)          # ACT: oth slice 0
            mult(h_mine_t, 0)                   # DVE: early mine mults
            mult(h_mine_t, 1)
            red_act(h_mine_t, s_mine, 0)        # ACT: mine slices 0..1
            red_act(h_mine_t, s_mine, 1)
            red_dve(h_oth_t, s_oth, 1)          # DVE: oth slices 1..3
            red_dve(h_oth_t, s_oth, 2)
            red_dve(h_oth_t, s_oth, 3)
            # oth half's exp + row-sum + its PE pass overlap mine's dots
            nc.scalar.activation(
                out=jnk_e[:, :], in_=s_oth[:, :], func=Exp,
                accum_out=rs_o[:, 0:1],
            )
            rs_o2 = spool.tile([P, 1], f32)
            nc.vector.tensor_copy(rs_o2[:, 0:1], rs_o[:, 0:1])
            nc.tensor.matmul(
                tot_psum[:, 0:1], ones_k[:, 0:P], rs_o2[:, 0:1],
                start=True, stop=False,
            )
            mult(h_mine_t, 2)
            mult(h_mine_t, 3)
            red_dve(h_mine_t, s_mine, 2)        # DVE: mine slices 2..3
            red_dve(h_mine_t, s_mine, 3)

            # --- tail: exp(mine), second PE pass, reciprocal, p ---
            nc.scalar.activation(
                out=e_mine[:, :], in_=s_mine[:, :], func=Exp,
                accum_out=rs_m[:, 0:1],
            )
            rs_m2 = spool.tile([P, 1], f32)
            nc.vector.tensor_copy(rs_m2[:, 0:1], rs_m[:, 0:1])
            nc.tensor.matmul(
                tot_psum[:, 0:1], ones_k[:, 0:P], rs_m2[:, 0:1],
                start=False, stop=True,
            )
            inv = spool.tile([P, 1], f32)
            nc.vector.reciprocal(inv[:, 0:1], tot_psum[:, 0:1])
            # p = e * (1/S): f32 for DVE fills' scalar operand, bf16 for
            # ACT fills' broadcast source
            p_f32 = spool.tile([P, RPP], f32)
            nc.vector.tensor_scalar_mul(p_f32[:, :], e_mine[:, :], inv[:, 0:1])
            p_bf = spool.tile([P, RPP], bf16)
            nc.vector.tensor_copy(p_bf[:, :], p_f32[:, :])

            # --- output: group g is p[:, g] broadcast along 4096 columns.
            # DVE fills: ones_4k * p (tensor_scalar, real strides);
            # ACT fills: stride-0 broadcast copy of p_bf. First and last
            # groups go out as two half-column pieces (earlier stream
            # start / smaller final completion receipt). All output DMAs
            # ride the sync ring. ---
            def fill(dst, g, eng):
                if eng == "v":
                    nc.vector.tensor_scalar_mul(
                        dst, ones_4k[:, 0 : dst.shape[-1]], p_f32[:, g : g + 1]
                    )
                else:
                    nc.scalar.copy(
                        out=dst,
                        in_=p_bf[:, g : g + 1].broadcast_to(
                            [P, dst.shape[-1]]
                        ),
                    )

            JH = N // 2

            def dma_half(ot, g, piece):
                j0 = piece * JH
                nc.sync.dma_start(
                    out=out_r[:, g : g + 1, j0 : j0 + JH],
                    in_=ot[:, j0 : j0 + JH].rearrange(
                        "q (r j) -> q r j", r=1
                    ),
                )

            # group 0 in halves (DVE fills)
            ot0 = opool.tile([P, SUP * N], bf16, tag="ot")
            fill(ot0[:, 0:JH], 0, "v")
            dma_half(ot0, 0, 0)
            fill(ot0[:, JH:N], 0, "v")
            dma_half(ot0, 0, 1)
            # groups 1..14 in supertiles of 2 (odd ACT, even DVE)
            gi = 1
            while gi + SUP <= RPP - 1:
                ot = opool.tile([P, SUP * N], bf16, tag="ot")
                for g in range(SUP):
                    fill(ot[:, g * N : (g + 1) * N], gi + g,
                         "a" if (gi + g) % 2 else "v")
                nc.sync.dma_start(
                    out=out_r[:, gi : gi + SUP, :],
                    in_=ot[:, 0 : SUP * N].rearrange(
                        "q (r j) -> q r j", r=SUP
                    ),
                )
                gi += SUP
            # last group (15) in halves (ACT fills)
            otl = opool.tile([P, SUP * N], bf16, tag="ot")
            fill(otl[:, 0:JH], RPP - 1, "a")
            dma_half(otl, RPP - 1, 0)
            fill(otl[:, JH:N], RPP - 1, "a")
            dma_half(otl, RPP - 1, 1)
    nc.compile()
    return nc


def _get_nc():
    if "nc" not in _CACHE:
        _CACHE["nc"] = _build()
    return _CACHE["nc"]


def _ensure_axon_hooks():
    """bass_utils' trace path imports antenv.axon_hooks, which some images
    lack; provide a stub and register the real NTFF hook (via the boot
    shim's ctypes path) so tracing works instead of degrading."""
    try:
        import antenv.axon_hooks  # noqa: F401
        return
    except ImportError:
        pass
    import sys
    import types

    try:
        import antenv
    except ImportError:
        antenv = types.ModuleType("antenv")
        sys.modules["antenv"] = antenv
    m = types.ModuleType("antenv.axon_hooks")
    m._hook = None
    m.set_axon_ntff_profile_hook = lambda h: setattr(m, "_hook", h)
    m.get_axon_ntff_profile_hook = lambda: m._hook
    sys.modules["antenv.axon_hooks"] = m
    try:
        from trn_agent_boot.trn_boot import _ntff_profile_via_ctypes

        hk = _ntff_profile_via_ctypes("/opt/axon/libaxon_pjrt.so")
        if hk is not None:
            m._hook = hk
    except Exception:
        pass


def run_on_device(h, w, trace=False):
    """Run the SPMD kernel; returns the BassKernelResults."""
    from concourse.bass_utils import run_bass_kernel_spmd

    _ensure_axon_hooks()

    in_maps = []
    for c in range(NCORES):
        b_idx, rh = divmod(c, 2)
        hb = h[b_idx]
        if rh:
            hb = np.concatenate([hb[HALF:], hb[:HALF]], axis=0)
        in_maps.append(
            {
                "h": np.ascontiguousarray(hb.astype(ml_dtypes.bfloat16)),
                "wb": np.ascontiguousarray(
                    np.broadcast_to(w.astype(ml_dtypes.bfloat16), (P, SL, D))
                ),
            }
        )
    res = run_bass_kernel_spmd(
        _get_nc(), in_maps, core_ids=list(range(NCORES)), trace=trace
    )
    return res


def kernel(h, w, b):
    h = np.asarray(h, dtype=np.float32)
    w = np.asarray(w, dtype=np.float32)
    res = run_on_device(h, w)
    A = np.empty((B, N, N), dtype=np.float32)
    for c in range(NCORES):
        b_idx, rh = divmod(c, 2)
        out_c = res.results[c]["out"]
        # device row 128r + q holds the value for local input row 16q + r:
        # undo with reshape/transpose (bf16 -> f32 upcast on assignment)
        unperm = (
            out_c.reshape(RPP, P, N).transpose(1, 0, 2).reshape(HALF, N)
        )
        A[b_idx, rh * HALF : (rh + 1) * HALF, :] = unperm
    return A
